# revision 24
# baseline (speedup 1.0000x reference)
"""Trainium2 Bass kernel for the spike-decoder GNN message-passing module.

Math (per batch b, output time tau in [0, T-2], variable v):
  out[b,tau,v] = bias[v]
               + sum_{i,k} w[v,i,k] * x[b,i,tau+k-(K-2)]          (static conv)
               + sum_{e: recv[e]=v} sum_k dw[e,b,tau,k] * x[b,send[e],tau+k-(K-2)]
with w = conv_weight masked at w[i,i,K-1] = 0, x = spikes[...,0] transposed to
[b, nvar, t], and out-of-range x treated as zero.

Sharding: 8 cores = (b in 0..3) x (time half h in 0..1). Each core computes a
1024-wide tau window ([0,1024) or [1023,2047) — one overlapping column keeps
shapes uniform for SPMD). dyn_weights is the memory-bound stream; it is cast
to bf16 on the host (tolerance 2e-2 dwarfs bf16's ~0.4% relative error),
halving the dominant HBM traffic to ~16.8 MB/core.

On-core algorithm:
  - xg[e,:] = x[send[e],:] is gathered on the HOST (pure indexing, like the
    one-hot recv matrix) and uploaded as bf16; a one-element-shifted copy
    xg_odd is made on ScalarE so every DVE sliding-window read starts 4B-
    aligned (bf16 2x perf mode needs aligned stride-{1,2} APs).
  - the dw stream is laid out by the host as 32 parity blocks per core
    (unit u = 8 consecutive ks of one (h2, et) tile; block A = even ks,
    block B = odd ks, each [128, 4*512] bf16 = 512 KB). Each block is one
    DMA and gates exactly one DVE tensor_mul — fine-grained DMA->DVE->PE
    pipelining with ~1.3 us per stage.
  - products P[e, m*CHUNK+tau] = dw_block * window(xg) on DVE (2x bf16).
  - k-reduction + recv-scatter + transpose folded into PE: per product
    column block, a matmul with stationary one-hot recvT accumulating into
    PSUM[v, tau].
  - static conv: 16 matmuls per tau-chunk with stationary wT_k (bf16) and
    shifted xpad slices (parity copies for alignment), interleaved into PE
    gaps at unit boundaries mid-stream.
  - bias: added by ScalarE during the PSUM -> SBUF copy (activation bias AP).
Output is [v, tau] per core; host transposes while assembling the result.
"""

import numpy as np

B, T, NVAR, K, E = 4, 2048, 128, 16, 512
TAU = T - 1            # 2047
L = 1024               # per-core tau window
NC_COUNT = 8
W_XPAD = L + K         # 1040
ETILES = E // 128      # 4
CHUNK = 512            # tau chunk per PSUM bank
NCHUNK = L // CHUNK    # 2
KH = K // 2            # 8 ks per half-tile unit
KQ = KH // 2           # 4 ks per parity block
BLK = KQ * CHUNK       # 2048 product columns per parity block
NT = NCHUNK * ETILES   # 8 dw tiles (et within h2)
NU = NT * 2            # 16 half-tile pipeline units

# host-side k reordering within each 8-k half: evens then odds
K_ORDER = [0, 2, 4, 6, 1, 3, 5, 7, 8, 10, 12, 14, 9, 11, 13, 15]

# Blocks (of 64 = 2*NU parity blocks) that ship as RAW fp8 over HWDGE and
# multiply at DVE 1x: halves those blocks' SBUF-write bytes, trading fabric
# time (the stream bottleneck) for idle DVE cycles. Spread mid-stream, away
# from the warm-up head and the latency-critical tail.
FP8_RAW_BLOCKS = (10, 16, 22, 26)

# dw ships as fp8 e3m4, scaled by DW_SCALE to center the distribution in the
# normal range (sigma 0.02*64 = 1.3, max ~8 << 15.5). The recv one-hot matrix
# carries 1/DW_SCALE (exact in bf16), so the scatter matmul unscales for free.
DW_SCALE = 64.0

_PROGRAM = None


def _build_program():
    import concourse.bass as bass
    import concourse.bacc as bacc
    import concourse.mybir as mybir
    import concourse.tile as tile

    f32 = mybir.dt.float32
    bf16 = mybir.dt.bfloat16
    fp8 = mybir.dt.float8e3  # e3m4; host scales dw by DW_SCALE to fit
    nc = bacc.Bacc()

    xpad_d = nc.declare_dram_parameter("xpad", [NVAR, W_XPAD], bf16, isOutput=False)
    xg_d = nc.declare_dram_parameter("xg", [E, W_XPAD], bf16, isOutput=False)
    dw_d = nc.declare_dram_parameter("dw", [NCHUNK * E, CHUNK * K], fp8, isOutput=False)
    wt_d = nc.declare_dram_parameter("wt", [NVAR, K * NVAR], bf16, isOutput=False)
    recv_d = nc.declare_dram_parameter("recvT", [128, ETILES * NVAR], bf16, isOutput=False)
    bias_d = nc.declare_dram_parameter("bias", [NVAR, 1], f32, isOutput=False)
    y_d = nc.declare_dram_parameter("yT", [NVAR, L], f32, isOutput=True)

    with tile.TileContext(nc) as tc:
        with (
            tc.tile_pool(name="consts", bufs=1) as consts,
            tc.tile_pool(name="dwp", bufs=16) as dwp,
            tc.tile_pool(name="prodp", bufs=10) as prodp,
            tc.tile_pool(name="opsum", bufs=2, space=bass.MemorySpace.PSUM) as opsum,
            tc.tile_pool(name="resp", bufs=2) as resp,
        ):
            xg_e = []
            xg_o = []
            for et in range(ETILES):
                xg_e.append(consts.tile([128, W_XPAD], bf16, name=f"xge{et}"))
                xg_o.append(consts.tile([128, W_XPAD], bf16, name=f"xgo{et}"))
            xpad_e = consts.tile([NVAR, W_XPAD], bf16)
            xpad_o = consts.tile([NVAR, W_XPAD], bf16)
            wt = consts.tile([NVAR, K * NVAR], bf16)
            recvT = consts.tile([128, ETILES * NVAR], bf16)
            biasT = consts.tile([NVAR, 1], f32)

            # 32 parity blocks (512 KB each as bf16); pool rotation provides
            # ~8 blocks of DMA runway ahead of compute.
            blk_tiles = []
            for bi in range(2 * NU):
                dt = fp8 if bi in FP8_RAW_BLOCKS else bf16
                blk_tiles.append(dwp.tile([128, BLK], dt, name="blk", tag="blk"))

            def blk_dma(u, par):
                bi = 2 * u + par
                ti, half = divmod(u, 2)
                h2, et = divmod(ti, ETILES)
                r0 = h2 * E + et * 128
                c0 = half * 2 * BLK + par * BLK
                src = dw_d[r0:r0 + 128, c0:c0 + BLK]
                if bi in FP8_RAW_BLOCKS:
                    # raw fp8 over HWDGE: half the SBUF-write bytes
                    nc.scalar.dma_start(blk_tiles[bi][:], src)
                else:
                    # SWDGE (gpsimd ring) casts fp8 -> bf16 inline during the
                    # transfer; it also keeps the bulk dw stream off the sync
                    # ring, so consts and dw start streaming concurrently.
                    nc.gpsimd.dma_start(blk_tiles[bi][:], src)

            # consts on the sync HWDGE ring (parallel to the dw stream);
            # xpad/wt last — their first consumer is the unit-5 statics.
            nc.sync.dma_start(recvT[:], recv_d[:])
            nc.sync.dma_start(xg_e[0][:], xg_d[0:128, :])
            nc.sync.dma_start(xg_e[1][:], xg_d[128:256, :])
            nc.sync.dma_start(xg_e[2][:], xg_d[256:384, :])
            nc.sync.dma_start(xg_e[3][:], xg_d[384:512, :])
            nc.sync.dma_start(biasT[:], bias_d[:])
            nc.sync.dma_start(xpad_e[:], xpad_d[:])
            nc.sync.dma_start(wt[:], wt_d[:])
            for u in range(NU):
                blk_dma(u, 0)
                blk_dma(u, 1)

            # Shifted copies for odd-k windows (ScalarE, otherwise idle):
            # xg_o[et][p, j] = xg_e[et][p, j+1]; same for xpad.
            for et in range(ETILES):
                nc.scalar.copy(xg_o[et][:, 0:W_XPAD - 1], xg_e[et][:, 1:W_XPAD])
            nc.scalar.copy(xpad_o[:, 0:W_XPAD - 1], xpad_e[:, 1:W_XPAD])

            ops_tiles = [
                opsum.tile([128, CHUNK], f32, name=f"ops{h2}", tag=f"ops{h2}")
                for h2 in range(NCHUNK)
            ]
            started = [False, False]

            def acc_mm(h2, lhsT, rhs, stop=False):
                st = not started[h2]
                started[h2] = True
                nc.tensor.matmul(ops_tiles[h2][:], lhsT, rhs, start=st, stop=stop)

            def static_mm(h2, k):
                t0 = h2 * CHUNK
                p = k & 1
                src = xpad_o if p else xpad_e
                acc_mm(h2, wt[:, k * NVAR:(k + 1) * NVAR],
                       src[:, t0 + k - p:t0 + k - p + CHUNK])

            # Static-conv schedule: into PE gaps at unit boundaries once
            # wt/xpad have arrived (queued behind the early blocks),
            # finishing before each half's copy-out.
            static_after = {u: [] for u in range(NU)}
            for i, k in enumerate(range(K)):
                static_after[5 + i % 2].append((0, k))
            for i, k in enumerate(range(K)):
                static_after[9 + i % 4].append((1, k))

            for u in range(NU):
                ti, half = divmod(u, 2)
                h2, et = divmod(ti, ETILES)
                t0 = h2 * CHUNK
                k0 = half * KH
                last_unit_of_h2 = (u == NT - 1 or u == NU - 1)
                for par, xsrc in ((0, xg_e[et]), (1, xg_o[et])):
                    blk = blk_tiles[2 * u + par]
                    brow = blk.tensor.shape[-1]
                    pt = prodp.tile([128, BLK], bf16, name="pt", tag="pt")
                    prow = pt.tensor.shape[-1]
                    xrow = xsrc.tensor.shape[-1]
                    in0 = bass.AP(blk.tensor, 0,
                                  [[brow, 128], [CHUNK, KQ], [1, CHUNK]])
                    # window: xsrc[p, t0 + k0 + 2m (+1 via xg_o) + tau]
                    in1 = bass.AP(xsrc.tensor, t0 + k0,
                                  [[xrow, 128], [2, KQ], [1, CHUNK]])
                    out3 = bass.AP(pt.tensor, 0,
                                   [[prow, 128], [CHUNK, KQ], [1, CHUNK]])
                    nc.vector.tensor_mul(out3, in0, in1)
                    # k-reduction + recv scatter on PE:
                    # psum[v,tau] += sum_e recvT[e,v] * P[e, m*CHUNK + tau]
                    for m in range(KQ):
                        rhs = bass.AP(pt.tensor, m * CHUNK,
                                      [[prow, 128], [1, CHUNK]])
                        acc_mm(h2, recvT[:, et * NVAR:(et + 1) * NVAR], rhs,
                               stop=(last_unit_of_h2 and par == 1
                                     and m == KQ - 1 and not static_after[u]))
                for h2s, k in static_after[u]:
                    static_mm(h2s, k)
                if last_unit_of_h2:
                    res = resp.tile([128, CHUNK], f32, name="res", tag="res")
                    # copy-out with bias folded in: res = ops + bias[v]
                    nc.scalar.add(res[:], ops_tiles[h2][:], biasT[:, 0:1])
                    nc.sync.dma_start(y_d[:, t0:t0 + CHUNK], res[:])

    nc.compile()
    return nc


def _get_program():
    global _PROGRAM
    if _PROGRAM is None:
        _PROGRAM = _build_program()
    return _PROGRAM


def _host_prep(spikes, conv_weight, conv_bias, dyn_weights, edge_send, edge_recv):
    import ml_dtypes

    spikes = np.asarray(spikes, dtype=np.float32)
    conv_weight = np.asarray(conv_weight, dtype=np.float32)
    conv_bias = np.asarray(conv_bias, dtype=np.float32)
    dyn_weights = np.asarray(dyn_weights, dtype=np.float32)
    edge_send = np.asarray(edge_send, dtype=np.int64)
    edge_recv = np.asarray(edge_recv, dtype=np.int64)

    x = np.ascontiguousarray(spikes[..., 0].transpose(0, 2, 1))  # [B, NVAR, T]

    recvT = np.zeros((128, ETILES * NVAR), ml_dtypes.bfloat16)
    for et in range(ETILES):
        rr = edge_recv[et * 128:(et + 1) * 128]
        recvT[np.arange(128), et * NVAR + rr] = 1.0 / DW_SCALE

    w = conv_weight.copy()
    w[np.arange(NVAR), np.arange(NVAR), K - 1] = 0.0
    wt = np.ascontiguousarray(w.transpose(1, 2, 0)).reshape(NVAR, K * NVAR)
    wt = wt.astype(ml_dtypes.bfloat16)

    bias_col = np.ascontiguousarray(conv_bias.reshape(NVAR, 1))

    in_maps = []
    for core in range(NC_COUNT):
        b, h = divmod(core, 2)
        tau0 = 0 if h == 0 else TAU - L  # 0 or 1023
        xpad = np.zeros((NVAR, W_XPAD), np.float32)
        lo = tau0 - (K - 2)  # first x column needed
        src_lo = max(lo, 0)
        xpad[:, src_lo - lo:W_XPAD - 1] = x[b, :, src_lo:tau0 + L + 1]
        xg = np.ascontiguousarray(xpad[edge_send, :]).astype(ml_dtypes.bfloat16)
        a = dyn_weights[:, b, tau0:tau0 + L, :]          # [E, L, K]
        a = a.reshape(E, NCHUNK, CHUNK, K)               # [E, h2, tau, k]
        a = a.transpose(1, 0, 3, 2)                      # [h2, E, k, tau]
        a = a[:, :, K_ORDER, :]                          # parity-block k order
        dw = np.ascontiguousarray(a).reshape(NCHUNK * E, CHUNK * K)
        dw = (dw * DW_SCALE).astype(ml_dtypes.float8_e3m4)
        in_maps.append({
            "xpad": xpad.astype(ml_dtypes.bfloat16),
            "xg": xg,
            "dw": dw,
            "wt": wt,
            "recvT": recvT,
            "bias": bias_col,
        })
    return in_maps


def _assemble(results):
    out = np.empty((B, TAU, NVAR, 1), np.float32)
    for core in range(NC_COUNT):
        b, h = divmod(core, 2)
        yT = results[core]["yT"]  # [NVAR, L]
        if h == 0:
            out[b, 0:L, :, 0] = yT.T
        else:
            out[b, L:TAU, :, 0] = yT[:, 1:L].T
    return out


def run_on_hw(in_maps, trace=False, **kwargs):
    from concourse.bass_utils import run_bass_kernel_spmd

    nc = _get_program()
    return run_bass_kernel_spmd(
        nc, in_maps, core_ids=list(range(NC_COUNT)), trace=trace, **kwargs
    )


def kernel(spikes, conv_weight, conv_bias, dyn_weights, edge_send, edge_recv):
    in_maps = _host_prep(
        spikes, conv_weight, conv_bias, dyn_weights, edge_send, edge_recv
    )
    res = run_on_hw(in_maps)
    return _assemble(res.results)


# revision 27
# speedup vs baseline: 1.1466x; 1.1466x over previous
"""Trainium2 Bass kernel for the spike-decoder GNN message-passing module.

Math (per batch b, output time tau in [0, T-2], variable v):
  out[b,tau,v] = bias[v]
               + sum_{i,k} w[v,i,k] * x[b,i,tau+k-(K-2)]          (static conv)
               + sum_{e: recv[e]=v} sum_k dw[e,b,tau,k] * x[b,send[e],tau+k-(K-2)]
with w = conv_weight masked at w[i,i,K-1] = 0, x = spikes[...,0] transposed to
[b, nvar, t], and out-of-range x treated as zero.

Sharding: 8 cores = (b in 0..3) x (time half h in 0..1). Each core computes a
1024-wide tau window ([0,1024) or [1023,2047) — one overlapping column keeps
shapes uniform for SPMD). dyn_weights is the memory-bound stream; it is cast
to bf16 on the host (tolerance 2e-2 dwarfs bf16's ~0.4% relative error),
halving the dominant HBM traffic to ~16.8 MB/core.

On-core algorithm:
  - xg[e,:] = x[send[e],:] is gathered on the HOST (pure indexing, like the
    one-hot recv matrix) and uploaded as bf16; a one-element-shifted copy
    xg_odd is made on ScalarE so every DVE sliding-window read starts 4B-
    aligned (bf16 2x perf mode needs aligned stride-{1,2} APs).
  - the dw stream is laid out by the host as 32 parity blocks per core
    (unit u = 8 consecutive ks of one (h2, et) tile; block A = even ks,
    block B = odd ks, each [128, 4*512] bf16 = 512 KB). Each block is one
    DMA and gates exactly one DVE tensor_mul — fine-grained DMA->DVE->PE
    pipelining with ~1.3 us per stage.
  - products P[e, m*CHUNK+tau] = dw_block * window(xg) on DVE (2x bf16).
  - k-reduction + recv-scatter + transpose folded into PE: per product
    column block, a matmul with stationary one-hot recvT accumulating into
    PSUM[v, tau].
  - static conv: 16 matmuls per tau-chunk with stationary wT_k (bf16) and
    shifted xpad slices (parity copies for alignment), interleaved into PE
    gaps at unit boundaries mid-stream.
  - bias: added by ScalarE during the PSUM -> SBUF copy (activation bias AP).
Output is [v, tau] per core; host transposes while assembling the result.
"""

import numpy as np

B, T, NVAR, K, E = 4, 2048, 128, 16, 512
TAU = T - 1            # 2047
L = 1024               # per-core tau window
NC_COUNT = 8
W_XPAD = L + K         # 1040
ETILES = E // 128      # 4
CHUNK = 512            # tau chunk per PSUM bank
NCHUNK = L // CHUNK    # 2
KH = K // 2            # 8 ks per half-tile unit
KQ = KH // 2           # 4 ks per parity block
BLK = KQ * CHUNK       # 2048 product columns per parity block
NT = NCHUNK * ETILES   # 8 dw tiles (et within h2)
NU = NT * 2            # 16 half-tile pipeline units

# host-side k reordering within each 8-k half: evens then odds
K_ORDER = [0, 2, 4, 6, 1, 3, 5, 7, 8, 10, 12, 14, 9, 11, 13, 15]

# Blocks (of 64 = 2*NU parity blocks) that ship as RAW fp8 over HWDGE and
# multiply at DVE 1x: halves those blocks' SBUF-write bytes, trading fabric
# time (the stream bottleneck) for idle DVE cycles. Spread mid-stream, away
# from the warm-up head and the latency-critical tail.
FP8_RAW_BLOCKS = ()

# dw ships as fp8 e3m4, scaled by DW_SCALE to center the distribution in the
# normal range (sigma 0.02*64 = 1.3, max ~8 << 15.5). The recv one-hot matrix
# carries 1/DW_SCALE (exact in bf16), so the scatter matmul unscales for free.
DW_SCALE = 64.0

_PROGRAM = None


def _build_program():
    import concourse.bass as bass
    import concourse.bacc as bacc
    import concourse.mybir as mybir
    import concourse.tile as tile

    f32 = mybir.dt.float32
    bf16 = mybir.dt.bfloat16
    fp8 = mybir.dt.float8e3  # e3m4; host scales dw by DW_SCALE to fit
    nc = bacc.Bacc()

    xpad_d = nc.declare_dram_parameter("xpad", [NVAR, W_XPAD], bf16, isOutput=False)
    xg_d = nc.declare_dram_parameter("xg", [E, W_XPAD], bf16, isOutput=False)
    dw_d = nc.declare_dram_parameter("dw", [NCHUNK * E, CHUNK * K], fp8, isOutput=False)
    wt_d = nc.declare_dram_parameter("wt", [NVAR, K * NVAR], bf16, isOutput=False)
    recv_d = nc.declare_dram_parameter("recvT", [128, ETILES * NVAR], bf16, isOutput=False)
    bias_d = nc.declare_dram_parameter("bias", [NVAR, 1], f32, isOutput=False)
    y_d = nc.declare_dram_parameter("yT", [NVAR, L], f32, isOutput=True)

    with tile.TileContext(nc) as tc:
        with (
            tc.tile_pool(name="consts", bufs=1) as consts,
            tc.tile_pool(name="dwp", bufs=16) as dwp,
            tc.tile_pool(name="prodp", bufs=10) as prodp,
            tc.tile_pool(name="opsum", bufs=2, space=bass.MemorySpace.PSUM) as opsum,
            tc.tile_pool(name="resp", bufs=2) as resp,
        ):
            xg_e = []
            xg_o = []
            for et in range(ETILES):
                xg_e.append(consts.tile([128, W_XPAD], bf16, name=f"xge{et}"))
                xg_o.append(consts.tile([128, W_XPAD], bf16, name=f"xgo{et}"))
            xpad_e = consts.tile([NVAR, W_XPAD], bf16)
            xpad_o = consts.tile([NVAR, W_XPAD], bf16)
            wt = consts.tile([NVAR, K * NVAR], bf16)
            recvT = consts.tile([128, ETILES * NVAR], bf16)
            biasT = consts.tile([NVAR, 1], f32)

            # 32 parity blocks (512 KB each as bf16); pool rotation provides
            # ~8 blocks of DMA runway ahead of compute.
            blk_tiles = []
            for bi in range(2 * NU):
                dt = fp8 if bi in FP8_RAW_BLOCKS else bf16
                blk_tiles.append(dwp.tile([128, BLK], dt, name="blk", tag="blk"))

            def blk_dma(u, par):
                bi = 2 * u + par
                ti, half = divmod(u, 2)
                h2, et = divmod(ti, ETILES)
                r0 = h2 * E + et * 128
                c0 = half * 2 * BLK + par * BLK
                src = dw_d[r0:r0 + 128, c0:c0 + BLK]
                if bi in FP8_RAW_BLOCKS:
                    # raw fp8 over HWDGE: half the SBUF-write bytes
                    nc.scalar.dma_start(blk_tiles[bi][:], src)
                else:
                    # SWDGE (gpsimd ring) casts fp8 -> bf16 inline during the
                    # transfer; it also keeps the bulk dw stream off the sync
                    # ring, so consts and dw start streaming concurrently.
                    nc.gpsimd.dma_start(blk_tiles[bi][:], src)

            # consts on the sync HWDGE ring (parallel to the dw stream);
            # xpad/wt early so the statics can fill the PE during the slow
            # DMA ramp phase instead of stealing steady-state PE time.
            nc.sync.dma_start(recvT[:], recv_d[:])
            nc.sync.dma_start(xg_e[0][:], xg_d[0:128, :])
            nc.sync.dma_start(xpad_e[:], xpad_d[:])
            nc.sync.dma_start(wt[:], wt_d[:])
            nc.sync.dma_start(xg_e[1][:], xg_d[128:256, :])
            nc.sync.dma_start(xg_e[2][:], xg_d[256:384, :])
            nc.sync.dma_start(xg_e[3][:], xg_d[384:512, :])
            nc.sync.dma_start(biasT[:], bias_d[:])
            for u in range(NU):
                blk_dma(u, 0)
                blk_dma(u, 1)

            # Shifted copies for odd-k windows (ScalarE, otherwise idle):
            # xg_o[et][p, j] = xg_e[et][p, j+1]; same for xpad.
            for et in range(ETILES):
                nc.scalar.copy(xg_o[et][:, 0:W_XPAD - 1], xg_e[et][:, 1:W_XPAD])
            nc.scalar.copy(xpad_o[:, 0:W_XPAD - 1], xpad_e[:, 1:W_XPAD])

            ops_tiles = [
                opsum.tile([128, CHUNK], f32, name=f"ops{h2}", tag=f"ops{h2}")
                for h2 in range(NCHUNK)
            ]
            started = [False, False]

            def acc_mm(h2, lhsT, rhs, stop=False):
                st = not started[h2]
                started[h2] = True
                nc.tensor.matmul(ops_tiles[h2][:], lhsT, rhs, start=st, stop=stop)

            def static_mm(h2, k):
                t0 = h2 * CHUNK
                p = k & 1
                src = xpad_o if p else xpad_e
                acc_mm(h2, wt[:, k * NVAR:(k + 1) * NVAR],
                       src[:, t0 + k - p:t0 + k - p + CHUNK])

            # Static-conv schedule: into PE gaps at unit boundaries once
            # wt/xpad have arrived (queued behind the early blocks),
            # finishing before each half's copy-out.
            static_after = {u: [] for u in range(NU)}
            for i, k in enumerate(range(K)):
                static_after[2 + i % 2].append((0, k))
            for i, k in enumerate(range(K)):
                static_after[3 + i % 2].append((1, k))

            for u in range(NU):
                ti, half = divmod(u, 2)
                h2, et = divmod(ti, ETILES)
                t0 = h2 * CHUNK
                k0 = half * KH
                last_unit_of_h2 = (u == NT - 1 or u == NU - 1)
                for par, xsrc in ((0, xg_e[et]), (1, xg_o[et])):
                    blk = blk_tiles[2 * u + par]
                    brow = blk.tensor.shape[-1]
                    pt = prodp.tile([128, BLK], bf16, name="pt", tag="pt")
                    prow = pt.tensor.shape[-1]
                    xrow = xsrc.tensor.shape[-1]
                    in0 = bass.AP(blk.tensor, 0,
                                  [[brow, 128], [CHUNK, KQ], [1, CHUNK]])
                    # window: xsrc[p, t0 + k0 + 2m (+1 via xg_o) + tau]
                    in1 = bass.AP(xsrc.tensor, t0 + k0,
                                  [[xrow, 128], [2, KQ], [1, CHUNK]])
                    out3 = bass.AP(pt.tensor, 0,
                                   [[prow, 128], [CHUNK, KQ], [1, CHUNK]])
                    nc.vector.tensor_mul(out3, in0, in1)
                    # k-reduction + recv scatter on PE:
                    # psum[v,tau] += sum_e recvT[e,v] * P[e, m*CHUNK + tau]
                    for m in range(KQ):
                        rhs = bass.AP(pt.tensor, m * CHUNK,
                                      [[prow, 128], [1, CHUNK]])
                        acc_mm(h2, recvT[:, et * NVAR:(et + 1) * NVAR], rhs,
                               stop=(last_unit_of_h2 and par == 1
                                     and m == KQ - 1 and not static_after[u]))
                for h2s, k in static_after[u]:
                    static_mm(h2s, k)
                if last_unit_of_h2:
                    res = resp.tile([128, CHUNK], f32, name="res", tag="res")
                    # copy-out with bias folded in: res = ops + bias[v]
                    nc.scalar.add(res[:], ops_tiles[h2][:], biasT[:, 0:1])
                    nc.sync.dma_start(y_d[:, t0:t0 + CHUNK], res[:])

    nc.compile()
    return nc


def _get_program():
    global _PROGRAM
    if _PROGRAM is None:
        _PROGRAM = _build_program()
    return _PROGRAM


def _host_prep(spikes, conv_weight, conv_bias, dyn_weights, edge_send, edge_recv):
    import ml_dtypes

    spikes = np.asarray(spikes, dtype=np.float32)
    conv_weight = np.asarray(conv_weight, dtype=np.float32)
    conv_bias = np.asarray(conv_bias, dtype=np.float32)
    dyn_weights = np.asarray(dyn_weights, dtype=np.float32)
    edge_send = np.asarray(edge_send, dtype=np.int64)
    edge_recv = np.asarray(edge_recv, dtype=np.int64)

    x = np.ascontiguousarray(spikes[..., 0].transpose(0, 2, 1))  # [B, NVAR, T]

    recvT = np.zeros((128, ETILES * NVAR), ml_dtypes.bfloat16)
    for et in range(ETILES):
        rr = edge_recv[et * 128:(et + 1) * 128]
        recvT[np.arange(128), et * NVAR + rr] = 1.0 / DW_SCALE

    w = conv_weight.copy()
    w[np.arange(NVAR), np.arange(NVAR), K - 1] = 0.0
    wt = np.ascontiguousarray(w.transpose(1, 2, 0)).reshape(NVAR, K * NVAR)
    wt = wt.astype(ml_dtypes.bfloat16)

    bias_col = np.ascontiguousarray(conv_bias.reshape(NVAR, 1))

    in_maps = []
    for core in range(NC_COUNT):
        b, h = divmod(core, 2)
        tau0 = 0 if h == 0 else TAU - L  # 0 or 1023
        xpad = np.zeros((NVAR, W_XPAD), np.float32)
        lo = tau0 - (K - 2)  # first x column needed
        src_lo = max(lo, 0)
        xpad[:, src_lo - lo:W_XPAD - 1] = x[b, :, src_lo:tau0 + L + 1]
        xg = np.ascontiguousarray(xpad[edge_send, :]).astype(ml_dtypes.bfloat16)
        a = dyn_weights[:, b, tau0:tau0 + L, :]          # [E, L, K]
        a = a.reshape(E, NCHUNK, CHUNK, K)               # [E, h2, tau, k]
        a = a.transpose(1, 0, 3, 2)                      # [h2, E, k, tau]
        a = a[:, :, K_ORDER, :]                          # parity-block k order
        dw = np.ascontiguousarray(a).reshape(NCHUNK * E, CHUNK * K)
        dw = (dw * DW_SCALE).astype(ml_dtypes.float8_e3m4)
        in_maps.append({
            "xpad": xpad.astype(ml_dtypes.bfloat16),
            "xg": xg,
            "dw": dw,
            "wt": wt,
            "recvT": recvT,
            "bias": bias_col,
        })
    return in_maps


def _assemble(results):
    out = np.empty((B, TAU, NVAR, 1), np.float32)
    for core in range(NC_COUNT):
        b, h = divmod(core, 2)
        yT = results[core]["yT"]  # [NVAR, L]
        if h == 0:
            out[b, 0:L, :, 0] = yT.T
        else:
            out[b, L:TAU, :, 0] = yT[:, 1:L].T
    return out


def run_on_hw(in_maps, trace=False, **kwargs):
    from concourse.bass_utils import run_bass_kernel_spmd

    nc = _get_program()
    return run_bass_kernel_spmd(
        nc, in_maps, core_ids=list(range(NC_COUNT)), trace=trace, **kwargs
    )


def kernel(spikes, conv_weight, conv_bias, dyn_weights, edge_send, edge_recv):
    in_maps = _host_prep(
        spikes, conv_weight, conv_bias, dyn_weights, edge_send, edge_recv
    )
    res = run_on_hw(in_maps)
    return _assemble(res.results)


# revision 33
# speedup vs baseline: 1.1886x; 1.0367x over previous
"""Trainium2 Bass kernel for the spike-decoder GNN message-passing module.

Math (per batch b, output time tau in [0, T-2], variable v):
  out[b,tau,v] = bias[v]
               + sum_{i,k} w[v,i,k] * x[b,i,tau+k-(K-2)]          (static conv)
               + sum_{e: recv[e]=v} sum_k dw[e,b,tau,k] * x[b,send[e],tau+k-(K-2)]
with w = conv_weight masked at w[i,i,K-1] = 0, x = spikes[...,0] transposed to
[b, nvar, t], and out-of-range x treated as zero.

Sharding: 8 cores = (b in 0..3) x (time half h in 0..1). Each core computes a
1024-wide tau window ([0,1024) or [1023,2047) — one overlapping column keeps
shapes uniform for SPMD). dyn_weights is the memory-bound stream; it is cast
to bf16 on the host (tolerance 2e-2 dwarfs bf16's ~0.4% relative error),
halving the dominant HBM traffic to ~16.8 MB/core.

On-core algorithm:
  - xg[e,:] = x[send[e],:] is gathered on the HOST (pure indexing, like the
    one-hot recv matrix) and uploaded as bf16; a one-element-shifted copy
    xg_odd is made on ScalarE so every DVE sliding-window read starts 4B-
    aligned (bf16 2x perf mode needs aligned stride-{1,2} APs).
  - the dw stream is laid out by the host as 32 parity blocks per core
    (unit u = 8 consecutive ks of one (h2, et) tile; block A = even ks,
    block B = odd ks, each [128, 4*512] bf16 = 512 KB). Each block is one
    DMA and gates exactly one DVE tensor_mul — fine-grained DMA->DVE->PE
    pipelining with ~1.3 us per stage.
  - products P[e, m*CHUNK+tau] = dw_block * window(xg) on DVE (2x bf16).
  - k-reduction + recv-scatter + transpose folded into PE: per product
    column block, a matmul with stationary one-hot recvT accumulating into
    PSUM[v, tau].
  - static conv: 16 matmuls per tau-chunk with stationary wT_k (bf16) and
    shifted xpad slices (parity copies for alignment), interleaved into PE
    gaps at unit boundaries mid-stream.
  - bias: added by ScalarE during the PSUM -> SBUF copy (activation bias AP).
Output is [v, tau] per core; host transposes while assembling the result.
"""

import numpy as np

B, T, NVAR, K, E = 4, 2048, 128, 16, 512
TAU = T - 1            # 2047
L = 1024               # per-core tau window
NC_COUNT = 8
W_XPAD = L + K         # 1040
ETILES = E // 128      # 4
CHUNK = 512            # tau chunk per PSUM bank
NCHUNK = L // CHUNK    # 2
KH = K // 2            # 8 ks per half-tile unit
KQ = KH // 2           # 4 ks per parity block
BLK = KQ * CHUNK       # 2048 product columns per parity block
NT = NCHUNK * ETILES   # 8 dw tiles (et within h2)
NU = NT * 2            # 16 half-tile pipeline units

# host-side k reordering within each 8-k half: evens then odds
K_ORDER = [0, 2, 4, 6, 1, 3, 5, 7, 8, 10, 12, 14, 9, 11, 13, 15]

# Blocks (of 64 = 2*NU parity blocks) that ship as RAW fp8 over HWDGE and
# multiply at DVE 1x: halves those blocks' SBUF-write bytes, trading fabric
# time (the stream bottleneck) for idle DVE cycles. Spread mid-stream, away
# from the warm-up head and the latency-critical tail.
FP8_RAW_BLOCKS = ()

# dw ships as fp8 e3m4, scaled by DW_SCALE to center the distribution in the
# normal range (sigma 0.02*64 = 1.3, max ~8 << 15.5). The recv one-hot matrix
# carries 1/DW_SCALE (exact in bf16), so the scatter matmul unscales for free.
DW_SCALE = 64.0

_PROGRAM = None


def _build_program():
    import concourse.bass as bass
    import concourse.bacc as bacc
    import concourse.mybir as mybir
    import concourse.tile as tile

    f32 = mybir.dt.float32
    bf16 = mybir.dt.bfloat16
    fp8 = mybir.dt.float8e3  # e3m4; host scales dw by DW_SCALE to fit
    nc = bacc.Bacc()

    # consts are packed into two bf16 blobs so the hw ring moves them in two
    # large transfers instead of eight small ones (each small DMA pays ~2 us
    # of serial latency during the ramp phase).
    # blob1 = [recvT(512) | xg0(1040)]; blob2 = [xg1|xg2|xg3|xpad|wt]
    B1W = ETILES * NVAR + W_XPAD                 # 1552
    B2W = 3 * W_XPAD + W_XPAD + K * NVAR         # 6208
    blob1_d = nc.declare_dram_parameter("blob1", [128, B1W], bf16, isOutput=False)
    blob2_d = nc.declare_dram_parameter("blob2", [128, B2W], bf16, isOutput=False)
    dw_d = nc.declare_dram_parameter("dw", [NCHUNK * E, CHUNK * K], fp8, isOutput=False)
    bias_d = nc.declare_dram_parameter("bias", [NVAR, 1], f32, isOutput=False)
    y_d = nc.declare_dram_parameter("yT", [NVAR, L], f32, isOutput=True)

    with tile.TileContext(nc) as tc:
        with (
            tc.tile_pool(name="consts", bufs=1) as consts,
            tc.tile_pool(name="dwp", bufs=16) as dwp,
            tc.tile_pool(name="prodp", bufs=10) as prodp,
            tc.tile_pool(name="opsum", bufs=2, space=bass.MemorySpace.PSUM) as opsum,
            tc.tile_pool(name="resp", bufs=2) as resp,
        ):
            blob1 = consts.tile([128, B1W], bf16, name="blob1")
            blob2 = consts.tile([128, B2W], bf16, name="blob2")
            # (tensor, column base) views into the blobs
            xg_e = [(blob1, ETILES * NVAR), (blob2, 0),
                    (blob2, W_XPAD), (blob2, 2 * W_XPAD)]
            XPAD_BASE = 3 * W_XPAD     # in blob2
            WT_BASE = 4 * W_XPAD       # in blob2
            xg_o = []
            for et in range(ETILES):
                xg_o.append(consts.tile([128, W_XPAD], bf16, name=f"xgo{et}"))
            xpad_o = consts.tile([NVAR, W_XPAD], bf16)
            biasT = consts.tile([NVAR, 1], f32)

            # 32 parity blocks (512 KB each as bf16); pool rotation provides
            # ~8 blocks of DMA runway ahead of compute.
            blk_tiles = []
            for bi in range(2 * NU):
                dt = fp8 if bi in FP8_RAW_BLOCKS else bf16
                blk_tiles.append(dwp.tile([128, BLK], dt, name="blk", tag="blk"))

            def blk_dma(u, par):
                bi = 2 * u + par
                ti, half = divmod(u, 2)
                h2, et = divmod(ti, ETILES)
                r0 = h2 * E + et * 128
                c0 = half * 2 * BLK + par * BLK
                src = dw_d[r0:r0 + 128, c0:c0 + BLK]
                if bi in FP8_RAW_BLOCKS:
                    # raw fp8 over HWDGE: half the SBUF-write bytes
                    nc.scalar.dma_start(blk_tiles[bi][:], src)
                else:
                    # SWDGE (gpsimd ring) casts fp8 -> bf16 inline during the
                    # transfer; it also keeps the bulk dw stream off the sync
                    # ring, so consts and dw start streaming concurrently.
                    nc.gpsimd.dma_start(blk_tiles[bi][:], src)

            # consts on the sync HWDGE ring (parallel to the dw stream)
            nc.sync.dma_start(blob1[:], blob1_d[:])
            nc.sync.dma_start(blob2[:], blob2_d[:])
            nc.sync.dma_start(biasT[:], bias_d[:])
            for u in range(NU):
                blk_dma(u, 0)
                blk_dma(u, 1)

            def view(pair, c0, w):
                t, base = pair
                row = t.tensor.shape[-1]
                return bass.AP(t.tensor, base + c0, [[row, 128], [1, w]])

            # Shifted copies for odd-k windows (ScalarE, otherwise idle):
            # xg_o[et][p, j] = xg_e[et][p, j+1]; same for xpad. (The slice
            # reads one column past its region at j=1039, which is never
            # consumed downstream.)
            for et in range(ETILES):
                nc.scalar.copy(xg_o[et][:, 0:W_XPAD - 1],
                               view(xg_e[et], 1, W_XPAD - 1))
            nc.scalar.copy(xpad_o[:, 0:W_XPAD - 1],
                           view((blob2, XPAD_BASE), 1, W_XPAD - 1))

            ops_tiles = [
                opsum.tile([128, CHUNK], f32, name=f"ops{h2}", tag=f"ops{h2}")
                for h2 in range(NCHUNK)
            ]
            started = [False, False]

            def acc_mm(h2, lhsT, rhs, stop=False):
                st = not started[h2]
                started[h2] = True
                nc.tensor.matmul(ops_tiles[h2][:], lhsT, rhs, start=st, stop=stop)

            def static_mm(h2, k):
                t0 = h2 * CHUNK
                p = k & 1
                if p:
                    rhs = xpad_o[:, t0 + k - 1:t0 + k - 1 + CHUNK]
                else:
                    rhs = view((blob2, XPAD_BASE), t0 + k, CHUNK)
                acc_mm(h2, view((blob2, WT_BASE), k * NVAR, NVAR), rhs)

            # Static-conv schedule: into PE gaps at unit boundaries once
            # wt/xpad have arrived (queued behind the early blocks),
            # finishing before each half's copy-out.
            static_after = {u: [] for u in range(NU)}
            for i, k in enumerate(range(K)):
                static_after[2 + i % 2].append((0, k))
            for i, k in enumerate(range(K)):
                static_after[3 + i % 2].append((1, k))

            for u in range(NU):
                ti, half = divmod(u, 2)
                h2, et = divmod(ti, ETILES)
                t0 = h2 * CHUNK
                k0 = half * KH
                last_unit_of_h2 = (u == NT - 1 or u == NU - 1)
                xe_t, xe_base = xg_e[et]
                for par, (xt, xbase) in ((0, (xe_t.tensor, xe_base)),
                                         (1, (xg_o[et].tensor, 0))):
                    blk = blk_tiles[2 * u + par]
                    brow = blk.tensor.shape[-1]
                    pt = prodp.tile([128, BLK], bf16, name="pt", tag="pt")
                    prow = pt.tensor.shape[-1]
                    xrow = xt.shape[-1]
                    in0 = bass.AP(blk.tensor, 0,
                                  [[brow, 128], [CHUNK, KQ], [1, CHUNK]])
                    # window: xsrc[p, t0 + k0 + 2m (+1 via xg_o) + tau]
                    in1 = bass.AP(xt, xbase + t0 + k0,
                                  [[xrow, 128], [2, KQ], [1, CHUNK]])
                    out3 = bass.AP(pt.tensor, 0,
                                   [[prow, 128], [CHUNK, KQ], [1, CHUNK]])
                    nc.vector.tensor_mul(out3, in0, in1)
                    # k-reduction + recv scatter on PE:
                    # psum[v,tau] += sum_e recvT[e,v] * P[e, m*CHUNK + tau]
                    for m in range(KQ):
                        rhs = bass.AP(pt.tensor, m * CHUNK,
                                      [[prow, 128], [1, CHUNK]])
                        acc_mm(h2, view((blob1, 0), et * NVAR, NVAR), rhs,
                               stop=(last_unit_of_h2 and par == 1
                                     and m == KQ - 1 and not static_after[u]))
                for h2s, k in static_after[u]:
                    static_mm(h2s, k)
                if last_unit_of_h2:
                    res = resp.tile([128, CHUNK], f32, name="res", tag="res")
                    # copy-out with bias folded in: res = ops + bias[v]
                    nc.scalar.add(res[:], ops_tiles[h2][:], biasT[:, 0:1])
                    nc.sync.dma_start(y_d[:, t0:t0 + CHUNK], res[:])

    nc.compile()
    return nc


def _get_program():
    global _PROGRAM
    if _PROGRAM is None:
        _PROGRAM = _build_program()
    return _PROGRAM


def _host_prep(spikes, conv_weight, conv_bias, dyn_weights, edge_send, edge_recv):
    import ml_dtypes

    spikes = np.asarray(spikes, dtype=np.float32)
    conv_weight = np.asarray(conv_weight, dtype=np.float32)
    conv_bias = np.asarray(conv_bias, dtype=np.float32)
    dyn_weights = np.asarray(dyn_weights, dtype=np.float32)
    edge_send = np.asarray(edge_send, dtype=np.int64)
    edge_recv = np.asarray(edge_recv, dtype=np.int64)

    x = np.ascontiguousarray(spikes[..., 0].transpose(0, 2, 1))  # [B, NVAR, T]

    recvT = np.zeros((128, ETILES * NVAR), ml_dtypes.bfloat16)
    for et in range(ETILES):
        rr = edge_recv[et * 128:(et + 1) * 128]
        recvT[np.arange(128), et * NVAR + rr] = 1.0 / DW_SCALE

    w = conv_weight.copy()
    w[np.arange(NVAR), np.arange(NVAR), K - 1] = 0.0
    wt = np.ascontiguousarray(w.transpose(1, 2, 0)).reshape(NVAR, K * NVAR)
    wt = wt.astype(ml_dtypes.bfloat16)

    bias_col = np.ascontiguousarray(conv_bias.reshape(NVAR, 1))

    in_maps = []
    for core in range(NC_COUNT):
        b, h = divmod(core, 2)
        tau0 = 0 if h == 0 else TAU - L  # 0 or 1023
        xpad = np.zeros((NVAR, W_XPAD), np.float32)
        lo = tau0 - (K - 2)  # first x column needed
        src_lo = max(lo, 0)
        xpad[:, src_lo - lo:W_XPAD - 1] = x[b, :, src_lo:tau0 + L + 1]
        xg = xpad[edge_send, :].astype(ml_dtypes.bfloat16)  # [E, W_XPAD]
        a = dyn_weights[:, b, tau0:tau0 + L, :]          # [E, L, K]
        a = a.reshape(E, NCHUNK, CHUNK, K)               # [E, h2, tau, k]
        a = a.transpose(1, 0, 3, 2)                      # [h2, E, k, tau]
        a = a[:, :, K_ORDER, :]                          # parity-block k order
        dw = np.ascontiguousarray(a).reshape(NCHUNK * E, CHUNK * K)
        dw = (dw * DW_SCALE).astype(ml_dtypes.float8_e3m4)
        blob1 = np.concatenate([recvT, xg[0:128]], axis=1)
        blob2 = np.concatenate(
            [xg[128:256], xg[256:384], xg[384:512],
             xpad.astype(ml_dtypes.bfloat16), wt], axis=1)
        in_maps.append({
            "blob1": np.ascontiguousarray(blob1),
            "blob2": np.ascontiguousarray(blob2),
            "dw": dw,
            "bias": bias_col,
        })
    return in_maps


def _assemble(results):
    out = np.empty((B, TAU, NVAR, 1), np.float32)
    for core in range(NC_COUNT):
        b, h = divmod(core, 2)
        yT = results[core]["yT"]  # [NVAR, L]
        if h == 0:
            out[b, 0:L, :, 0] = yT.T
        else:
            out[b, L:TAU, :, 0] = yT[:, 1:L].T
    return out


def run_on_hw(in_maps, trace=False, **kwargs):
    from concourse.bass_utils import run_bass_kernel_spmd

    nc = _get_program()
    return run_bass_kernel_spmd(
        nc, in_maps, core_ids=list(range(NC_COUNT)), trace=trace, **kwargs
    )


def kernel(spikes, conv_weight, conv_bias, dyn_weights, edge_send, edge_recv):
    in_maps = _host_prep(
        spikes, conv_weight, conv_bias, dyn_weights, edge_send, edge_recv
    )
    res = run_on_hw(in_maps)
    return _assemble(res.results)
